# revision 3
# baseline (speedup 1.0000x reference)
"""3-layer GAT (8 heads x 32 hid, PyG GATConv semantics w/ self-loops) +
global mean pool + linear, distributed over 8 Trainium2 NeuronCores.

Strategy (per sharding hint): nodes partitioned into 8 contiguous ranges
(dst-owner); incident edges live with their dst core, sorted by dst then src.
Each layer: node phase computes hh=[h|alpha_src] and alpha_dst for local
nodes, AllGather replicates hh; edge phase gathers hh[src] rows (indirect
DMA, 128 rows/instr), computes un-normalized attention numerator and
denominator with one-hot segment matmuls accumulated in PSUM, then divides
per dst node (segment softmax is algebraically folded: out = sum(ex*h)/sum(ex),
no max subtraction needed since e is O(1)-bounded). Per-graph mean-pool
partials are scattered into a 512-row buffer and AllReduced.

Self-contained: hardcodes the problem shapes; host-side preprocessing uses
only graph structure (edge_index, batch) and parameter repacking.
"""
import math
import numpy as np

import concourse.bass as bass
import concourse.bacc as bacc
import concourse.mybir as mybir
import concourse.tile as tile

P = 128
HEADS, HID = 8, 32
DH = HEADS * HID          # 256
DA = DH + HEADS           # 264 = h | alpha_src
DW = DH + 2 * HEADS       # 272 = W | W@Asrc | W@Adst
IN_CH = 128
NEG = 0.2
F32 = mybir.dt.float32
I32 = mybir.dt.int32


# ----------------------------------------------------------------- host prep

def host_prep(x, edge_index, batch, Ws, a_srcs, a_dsts, biases, lin_w, lin_b,
              N, E, G, ncores):
    """Build per-core input maps + the (core-uniform) program config."""
    nl_real = N // ncores
    assert nl_real * ncores == N
    NL = ((nl_real + P - 1) // P) * P          # padded local nodes
    NT = NL // P                               # dst tiles per core
    GP = ((G + P - 1) // P) * P                # padded graphs

    src = np.concatenate([edge_index[0], np.arange(N, dtype=np.int64)])
    dst = np.concatenate([edge_index[1], np.arange(N, dtype=np.int64)])
    core_of = dst // nl_real
    dloc_all = dst - core_of * nl_real

    # per-core, per-tile edge lists sorted by (dst_local, src)
    per_core = []
    counts = np.zeros((ncores, NT), np.int64)
    for k in range(ncores):
        m = core_of == k
        s_k, d_k = src[m], dloc_all[m]
        order = np.lexsort((s_k, d_k))
        s_k, d_k = s_k[order], d_k[order]
        t_k = d_k // P
        per_core.append((s_k, d_k, t_k))
        cnt = np.bincount(t_k, minlength=NT)
        counts[k] = cnt
        # pad nodes in the last tile get one dummy self-ish edge (den>0)
        counts[k, NT - 1] += NL - nl_real
    m_t = [int(x) for x in np.ceil(counts.max(axis=0) / P).astype(np.int64)]
    SM = int(sum(m_t))
    offs = np.concatenate([[0], np.cumsum(m_t)]).astype(np.int64)

    in_maps = []
    batch = np.asarray(batch, np.int64)
    # graph slot base per core
    for k in range(ncores):
        s_k, d_k, t_k = per_core[k]
        srcflat = np.zeros((P, SM), np.int64)
        dstflat = np.full((P, SM), 999.0, np.float32)
        tile_starts = np.searchsorted(t_k, np.arange(NT))
        tile_ends = np.searchsorted(t_k, np.arange(NT) + 1)
        for t in range(NT):
            lo, hi = tile_starts[t], tile_ends[t]
            ss = s_k[lo:hi]
            dd = d_k[lo:hi] - t * P
            if t == NT - 1 and NL > nl_real:
                npad = NL - nl_real
                ss = np.concatenate([ss, np.zeros(npad, np.int64)])
                dd = np.concatenate([dd, np.arange(nl_real - t * P,
                                                   nl_real - t * P + npad)])
            n = len(ss)
            cols = np.arange(n) // P + offs[t]
            rows = np.arange(n) % P
            gsrc = (ss // nl_real) * NL + (ss % nl_real)  # padded-global row
            srcflat[rows, cols] = gsrc
            dstflat[rows, cols] = dd.astype(np.float32)

        bl = batch[k * nl_real:(k + 1) * nl_real]
        g_base = int(bl[0])
        bshift = np.full(NL, 999.0, np.float32)
        bshift[:nl_real] = (bl - g_base).astype(np.float32)
        gslot = np.arange(g_base, g_base + P, dtype=np.int64)
        gslot = np.where(gslot < G, gslot, 99999).astype(np.int32)[:, None]

        xk = np.zeros((NL, IN_CH), np.float32)
        xk[:nl_real] = x[k * nl_real:(k + 1) * nl_real]

        im = {
            "x_local": xk,
            "srcidx": srcflat.reshape(-1).astype(np.int32),
            "dstloc": dstflat.reshape(-1).astype(np.float32),
            "bshift": bshift,
            "gslot": gslot,
        }
        in_maps.append(im)

    # replicated parameters
    def aug(W, a_s, a_d):
        cin = W.shape[0]
        As = np.zeros((DH, HEADS), np.float32)
        Ad = np.zeros((DH, HEADS), np.float32)
        for h in range(HEADS):
            As[h * HID:(h + 1) * HID, h] = a_s[h]
            Ad[h * HID:(h + 1) * HID, h] = a_d[h]
        return np.concatenate([W, W @ As, W @ Ad], axis=1).astype(np.float32)

    cnts = np.bincount(batch, minlength=G).astype(np.float32)
    invcnt = np.zeros((GP, 1), np.float32)
    invcnt[:G, 0] = 1.0 / np.maximum(cnts, 1.0)
    params = {
        "W0aug": aug(Ws[0], a_srcs[0], a_dsts[0]),
        "W1aug": aug(Ws[1], a_srcs[1], a_dsts[1]),
        "W2aug": aug(Ws[2], a_srcs[2], a_dsts[2]),
        "b0": np.tile(biases[0][None, :], (P, 1)).astype(np.float32),
        "b1": np.tile(biases[1][None, :], (P, 1)).astype(np.float32),
        "b2": np.tile(biases[2][None, :], (P, 1)).astype(np.float32),
        "linw": np.tile(lin_w.reshape(1, DH), (P, 1)).astype(np.float32),
        "linb": np.tile(lin_b.reshape(1, 1), (P, 1)).astype(np.float32),
        "invcnt": invcnt,
    }
    for im in in_maps:
        im.update(params)

    cfg = dict(NL=NL, NT=NT, GP=GP, m_t=tuple(m_t), SM=SM, ncores=ncores)
    return cfg, in_maps


# ------------------------------------------------------------- program build

def build_program(cfg, use_f32r=False):
    NL, NT, GP = cfg["NL"], cfg["NT"], cfg["GP"]
    m_t, SM, ncores = cfg["m_t"], cfg["SM"], cfg["ncores"]
    NG = NL * ncores                     # padded-global node rows
    MMDT = mybir.dt.float32r if use_f32r else F32

    nc = bacc.Bacc("TRN2", target_bir_lowering=False, debug=False,
                   num_devices=ncores)
    # ---------------- I/O
    x_in = nc.dram_tensor("x_local", [NL, IN_CH], F32, kind="ExternalInput")
    srcidx = nc.dram_tensor("srcidx", [P * SM], I32, kind="ExternalInput")
    dstloc = nc.dram_tensor("dstloc", [P * SM], F32, kind="ExternalInput")
    bshift = nc.dram_tensor("bshift", [NL], F32, kind="ExternalInput")
    gslot = nc.dram_tensor("gslot", [P, 1], I32, kind="ExternalInput")
    Waug = [nc.dram_tensor(f"W{l}aug", [IN_CH if l == 0 else DH, DW], F32,
                           kind="ExternalInput") for l in range(3)]
    bias = [nc.dram_tensor(f"b{l}", [P, DH], F32, kind="ExternalInput")
            for l in range(3)]
    linw = nc.dram_tensor("linw", [P, DH], F32, kind="ExternalInput")
    linb = nc.dram_tensor("linb", [P, 1], F32, kind="ExternalInput")
    invcnt = nc.dram_tensor("invcnt", [GP, 1], F32, kind="ExternalInput")
    y = nc.dram_tensor("y", [GP, 1], F32, kind="ExternalOutput")

    with tile.TileContext(nc) as tc:
        with tc.tile_pool(name="const", bufs=1) as cst, \
             tc.tile_pool(name="dram", bufs=1, space="DRAM") as dram, \
             tc.tile_pool(name="work", bufs=3) as wk, \
             tc.tile_pool(name="gpool", bufs=4) as gp, \
             tc.tile_pool(name="psA", bufs=2, space="PSUM") as psA, \
             tc.tile_pool(name="psB", bufs=2, space="PSUM") as psB, \
             tc.tile_pool(name="psN", bufs=2, space="PSUM") as psN, \
             tc.tile_pool(name="psP", bufs=1, space="PSUM") as psP:

            # ---------------- DRAM intermediates
            hh_local = dram.tile([NL, DA], F32)
            ad_local = dram.tile([NL, HEADS], F32)
            hh_full = dram.tile([NG, DA], F32)
            helu = [dram.tile([NL, DH], F32, tag=f"helu{i}", name=f"helu{i}")
                    for i in range(2)]
            pool_loc = dram.tile([GP, DH], F32)
            pool_sum = dram.tile([GP, DH], F32)

            # ---------------- constants
            ident = cst.tile([P, P], MMDT)
            iota_i = cst.tile([P, P], I32)
            nc.gpsimd.iota(iota_i[:], pattern=[[1, P]], base=0,
                           channel_multiplier=0)
            iota_f = cst.tile([P, P], F32)
            nc.vector.tensor_copy(iota_f[:], iota_i[:])
            iota_ci = cst.tile([P, 1], I32)
            nc.gpsimd.iota(iota_ci[:], pattern=[[0, 1]], base=0,
                           channel_multiplier=1)
            iota_cf = cst.tile([P, 1], F32)
            nc.vector.tensor_copy(iota_cf[:], iota_ci[:])
            nc.vector.tensor_tensor(out=ident[:],
                                    in0=iota_cf[:].to_broadcast([P, P]),
                                    in1=iota_f[:], op=mybir.AluOpType.is_equal)

            idx_all = cst.tile([P, SM], I32)
            nc.sync.dma_start(idx_all[:], srcidx[:].rearrange("(p j) -> p j", j=SM))
            dst_all = cst.tile([P, SM], F32)
            nc.sync.dma_start(dst_all[:], dstloc[:].rearrange("(p j) -> p j", j=SM))

            W_t = []
            for l in range(3):
                cin = IN_CH if l == 0 else DH
                tiles = []
                for kk in range(cin // P):
                    t = cst.tile([P, DW], MMDT, tag=f"W{l}_{kk}")
                    nc.sync.dma_start(t[:], Waug[l][kk * P:(kk + 1) * P, :])
                    tiles.append(t)
                W_t.append(tiles)
            bias_t = []
            for l in range(3):
                t = cst.tile([P, DH], F32, tag=f"bias{l}")
                nc.sync.dma_start(t[:], bias[l][:, :])
                bias_t.append(t)
            linw_t = cst.tile([P, DH], F32)
            nc.sync.dma_start(linw_t[:], linw[:, :])
            linb_t = cst.tile([P, 1], F32)
            nc.sync.dma_start(linb_t[:], linb[:, :])
            gslot_t = cst.tile([P, 1], I32)
            nc.sync.dma_start(gslot_t[:], gslot[:, :])

            # ---------------- phases
            def node_phase(l):
                """h_in (x or helu[l-1]) @ Waug_l -> hh_local, ad_local."""
                cin = IN_CH if l == 0 else DH
                src_d = x_in if l == 0 else helu[l - 1]
                for nt in range(NT):
                    in_t = wk.tile([P, cin], F32, tag="node_in")
                    nc.sync.dma_start(in_t[:], src_d[nt * P:(nt + 1) * P, :])
                    ps_o = psN.tile([P, DW], F32, space="PSUM", tag="node_mm")
                    for kk in range(cin // P):
                        trp = psB.tile([P, P + HEADS], MMDT, space="PSUM",
                                       tag="trp")
                        nc.tensor.transpose(out=trp[:, 0:P],
                                            in_=in_t[:, kk * P:(kk + 1) * P],
                                            identity=ident[:])
                        inT = wk.tile([P, P], MMDT, tag="node_inT")
                        nc.scalar.copy(inT[:], trp[:, 0:P])
                        nc.tensor.matmul(ps_o[:], lhsT=inT[:], rhs=W_t[l][kk][:],
                                         start=(kk == 0), stop=(kk == cin // P - 1))
                    hh_t = wk.tile([P, DW], F32, tag="node_hh")
                    nc.vector.tensor_copy(hh_t[:], ps_o[:])
                    nc.sync.dma_start(hh_local[nt * P:(nt + 1) * P, :],
                                      hh_t[:, 0:DA])
                    nc.sync.dma_start(ad_local[nt * P:(nt + 1) * P, :],
                                      hh_t[:, DA:DW])

            def all_gather_hh():
                nc.gpsimd.collective_compute(
                    "AllGather", mybir.AluOpType.bypass,
                    ins=[hh_local[:, :].opt()], outs=[hh_full[:, :].opt()],
                    replica_groups=[list(range(ncores))])

            def edge_phase(l):
                last = (l == 2)
                if last:
                    pool_ps = psP.tile([P, DH], F32, space="PSUM", tag="pool")
                off = 0
                for t in range(NT):
                    m = m_t[t]
                    ad_t = wk.tile([P, HEADS], F32, tag="ad")
                    nc.sync.dma_start(ad_t[:], ad_local[t * P:(t + 1) * P, :])
                    acc = psA.tile([P, DA], F32, space="PSUM", tag="acc")
                    for j in range(m):
                        c = off + j
                        g = gp.tile([P, DA], F32, tag="hhg")
                        nc.gpsimd.indirect_dma_start(
                            out=g[:, :], out_offset=None, in_=hh_full[:, :],
                            in_offset=bass.IndirectOffsetOnAxis(
                                ap=idx_all[:, c:c + 1], axis=0))
                        oh = wk.tile([P, P], MMDT, tag="oh")
                        nc.vector.tensor_tensor(
                            out=oh[:],
                            in0=dst_all[:, c:c + 1].to_broadcast([P, P]),
                            in1=iota_f[:], op=mybir.AluOpType.is_equal)
                        trp = psB.tile([P, P + HEADS], MMDT, space="PSUM",
                                       tag="trp")
                        nc.tensor.transpose(out=trp[:, 0:P], in_=oh[:],
                                            identity=ident[:])
                        ohT = wk.tile([P, P], MMDT, tag="ohT")
                        nc.scalar.copy(ohT[:], trp[:, 0:P])
                        nc.tensor.matmul(trp[:, P:P + HEADS], lhsT=ohT[:],
                                         rhs=ad_t[:], start=True, stop=True)
                        rhs_t = wk.tile([P, DA], MMDT, tag="rhs")
                        e_t = wk.tile([P, HEADS], F32, tag="e")
                        nc.vector.tensor_add(e_t[:], g[:, DH:DA],
                                             trp[:, P:P + HEADS])
                        nc.vector.scalar_tensor_tensor(
                            out=e_t[:], in0=e_t[:], scalar=NEG, in1=e_t[:],
                            op0=mybir.AluOpType.mult, op1=mybir.AluOpType.max)
                        nc.scalar.activation(rhs_t[:, DH:DA], e_t[:],
                                             mybir.ActivationFunctionType.Exp)
                        nc.vector.tensor_mul(
                            rhs_t[:, 0:DH].rearrange("p (h c) -> p h c", h=HEADS),
                            g[:, 0:DH].rearrange("p (h c) -> p h c", h=HEADS),
                            rhs_t[:, DH:DA][:, :, None].to_broadcast(
                                [P, HEADS, HID]))
                        nc.tensor.matmul(acc[:], lhsT=oh[:], rhs=rhs_t[:],
                                         start=(j == 0), stop=(j == m - 1))
                    off += m
                    # epilogue: out = elu(num/den + bias)
                    inv_t = wk.tile([P, HEADS], F32, tag="inv")
                    nc.vector.reciprocal(inv_t[:], acc[:, DH:DA])
                    h0 = wk.tile([P, DH], F32, tag="h0")
                    nc.vector.tensor_mul(
                        h0[:].rearrange("p (h c) -> p h c", h=HEADS),
                        acc[:, 0:DH].rearrange("p (h c) -> p h c", h=HEADS),
                        inv_t[:, :, None].to_broadcast([P, HEADS, HID]))
                    nc.vector.tensor_add(h0[:], h0[:], bias_t[l][:])
                    tm = wk.tile([P, DH], F32, tag="tm")
                    nc.vector.tensor_scalar_min(tm[:], h0[:], 0.0)
                    nc.scalar.activation(tm[:], tm[:],
                                         mybir.ActivationFunctionType.Exp)
                    out_t = wk.tile([P, DH], F32, tag="hout")
                    nc.vector.scalar_tensor_tensor(
                        out=out_t[:], in0=h0[:], scalar=0.0, in1=tm[:],
                        op0=mybir.AluOpType.max, op1=mybir.AluOpType.add)
                    nc.vector.tensor_scalar_add(out_t[:], out_t[:], -1.0)
                    if not last:
                        nc.sync.dma_start(helu[l][t * P:(t + 1) * P, :], out_t[:])
                    else:
                        gcol = wk.tile([P, 1], F32, tag="gcol")
                        nc.sync.dma_start(gcol[:], bshift[t * P:(t + 1) * P, None])
                        ohp = wk.tile([P, P], MMDT, tag="ohp")
                        nc.vector.tensor_tensor(
                            out=ohp[:], in0=gcol[:, 0:1].to_broadcast([P, P]),
                            in1=iota_f[:], op=mybir.AluOpType.is_equal)
                        nc.tensor.matmul(pool_ps[:], lhsT=ohp[:], rhs=out_t[:],
                                         start=(t == 0), stop=(t == NT - 1))

                if last:
                    # zero pool_loc then scatter local slots
                    zt = wk.tile([P, DH], F32, tag="zero")
                    nc.gpsimd.memset(zt[:], 0.0)
                    for b in range(GP // P):
                        nc.sync.dma_start(pool_loc[b * P:(b + 1) * P, :], zt[:])
                    pl = wk.tile([P, DH], F32, tag="plocal")
                    nc.vector.tensor_copy(pl[:], pool_ps[:])
                    nc.gpsimd.indirect_dma_start(
                        out=pool_loc[:, :],
                        out_offset=bass.IndirectOffsetOnAxis(
                            ap=gslot_t[:, 0:1], axis=0),
                        in_=pl[:, :], in_offset=None,
                        bounds_check=GP - 1, oob_is_err=False)

            # ---------------- run the layers
            for l in range(3):
                node_phase(l)
                all_gather_hh()
                edge_phase(l)

            nc.gpsimd.collective_compute(
                "AllReduce", mybir.AluOpType.add,
                ins=[pool_loc[:, :].opt()], outs=[pool_sum[:, :].opt()],
                replica_groups=[list(range(ncores))])

            # final linear: y = (pool_sum * invcnt) @ lin_w + lin_b
            for b in range(GP // P):
                pt = wk.tile([P, DH], F32, tag="psum_t")
                nc.sync.dma_start(pt[:], pool_sum[b * P:(b + 1) * P, :])
                ic = wk.tile([P, 1], F32, tag="ic")
                nc.sync.dma_start(ic[:], invcnt[b * P:(b + 1) * P, :])
                mulw = wk.tile([P, DH], F32, tag="mulw")
                nc.vector.tensor_mul(mulw[:], pt[:], linw_t[:])
                rs = wk.tile([P, 1], F32, tag="rs")
                nc.vector.reduce_sum(rs[:], mulw[:], axis=mybir.AxisListType.X)
                nc.vector.tensor_mul(rs[:], rs[:], ic[:])
                nc.vector.tensor_add(rs[:], rs[:], linb_t[:])
                nc.sync.dma_start(y[b * P:(b + 1) * P, :], rs[:])

    nc.compile()
    return nc


# ------------------------------------------------------------------- runner

class SpmdRunner:
    def __init__(self, nc, n_cores):
        import jax
        from jax.sharding import Mesh, PartitionSpec
        from jax.experimental.shard_map import shard_map
        from concourse.bass2jax import (
            _bass_exec_p, install_neuronx_cc_hook, partition_id_tensor)
        self.jax = jax
        install_neuronx_cc_hook()
        self.nc = nc
        self.n_cores = n_cores
        partition_name = (nc.partition_id_tensor.name
                          if nc.partition_id_tensor else None)
        in_names, out_names, out_avals, zero_outs = [], [], [], []
        for alloc in nc.m.functions[0].allocations:
            if not isinstance(alloc, mybir.MemoryLocationSet):
                continue
            name = alloc.memorylocations[0].name
            if alloc.kind == "ExternalInput":
                if name != partition_name and name != (
                        nc.dbg_addr.name if nc.dbg_addr else None):
                    in_names.append(name)
            elif alloc.kind == "ExternalOutput":
                out_names.append(name)
                shape = tuple(alloc.tensor_shape)
                dtype = mybir.dt.np(alloc.dtype)
                out_avals.append(jax.core.ShapedArray(shape, dtype))
                zero_outs.append(np.zeros(shape, dtype))
        self.in_names, self.out_names = in_names, out_names
        self.out_avals, self.zero_outs = out_avals, zero_outs
        n_params = len(in_names)
        all_in_names = list(in_names) + list(out_names)
        has_dbg = nc.dbg_addr is not None
        if has_dbg:
            all_in_names.append(nc.dbg_addr.name)
        if partition_name is not None:
            all_in_names.append(partition_name)

        def _body(*args):
            operands = list(args)
            if has_dbg:
                operands.append(jax.numpy.zeros((1, 2), jax.numpy.uint32))
            if partition_name is not None:
                operands.append(partition_id_tensor())
            outs = _bass_exec_p.bind(
                *operands, out_avals=tuple(out_avals),
                in_names=tuple(all_in_names), out_names=tuple(out_names),
                lowering_input_output_aliases=(),
                sim_require_finite=False, sim_require_nnan=False, nc=nc)
            return tuple(outs)

        devices = jax.devices()[:n_cores]
        assert len(devices) == n_cores
        mesh = Mesh(np.asarray(devices), ("core",))
        in_specs = (PartitionSpec("core"),) * (n_params + len(out_names))
        out_specs = (PartitionSpec("core"),) * len(out_names)
        self.fn = jax.jit(
            shard_map(_body, mesh=mesh, in_specs=in_specs,
                      out_specs=out_specs, check_rep=False),
            keep_unused=True)

    def prepare(self, in_maps):
        per_core = [[np.ascontiguousarray(m[nm]) for nm in self.in_names]
                    for m in in_maps]
        concat_in = [
            np.concatenate([per_core[c][i] for c in range(self.n_cores)], axis=0)
            for i in range(len(self.in_names))]
        concat_zero = [
            np.zeros((self.n_cores * z.shape[0], *z.shape[1:]), z.dtype)
            for z in self.zero_outs]
        args = [self.jax.device_put(a) for a in concat_in + concat_zero]
        for a in args:
            a.block_until_ready()
        return args

    def run(self, args):
        outs = self.fn(*args)
        self.jax.block_until_ready(outs)
        return outs

    def results(self, outs):
        res = []
        for c in range(self.n_cores):
            m = {}
            for i, nm in enumerate(self.out_names):
                m[nm] = np.asarray(outs[i]).reshape(
                    self.n_cores, *self.out_avals[i].shape)[c]
            res.append(m)
        return res


# -------------------------------------------------------------------- kernel

_CACHE = {}

N_FULL, E_FULL, G_FULL, NCORES = 50000, 800000, 512, 8
USE_F32R = False


def kernel(x, edge_index, batch,
           W0, a_src0, a_dst0, bias0,
           W1, a_src1, a_dst1, bias1,
           W2, a_src2, a_dst2, bias2,
           lin_w, lin_b):
    x = np.asarray(x, np.float32)
    edge_index = np.asarray(edge_index, np.int64)
    batch = np.asarray(batch, np.int64)
    N, E, G = x.shape[0], edge_index.shape[1], G_FULL

    cfg, in_maps = host_prep(
        x, edge_index, batch,
        [np.asarray(W0, np.float32), np.asarray(W1, np.float32),
         np.asarray(W2, np.float32)],
        [np.asarray(a_src0, np.float32), np.asarray(a_src1, np.float32),
         np.asarray(a_src2, np.float32)],
        [np.asarray(a_dst0, np.float32), np.asarray(a_dst1, np.float32),
         np.asarray(a_dst2, np.float32)],
        [np.asarray(bias0, np.float32), np.asarray(bias1, np.float32),
         np.asarray(bias2, np.float32)],
        np.asarray(lin_w, np.float32), np.asarray(lin_b, np.float32),
        N, E, G, NCORES)

    key = (cfg["NL"], cfg["NT"], cfg["GP"], cfg["m_t"], cfg["SM"],
           cfg["ncores"], USE_F32R)
    if key not in _CACHE:
        nc = build_program(cfg, use_f32r=USE_F32R)
        _CACHE[key] = (nc, SpmdRunner(nc, NCORES))
    nc, runner = _CACHE[key]

    args = runner.prepare(in_maps)
    outs = runner.run(args)
    res = runner.results(outs)
    return res[0]["y"][:G].astype(np.float32)
